# revision 29
# baseline (speedup 1.0000x reference)
"""Trainium2 Bass kernel for nn_AttentionBlock (B=4, C=64, H=W=64, INTER=8).

Sharding: 8 cores = 4 batches x 2 query-halves. Each core computes, for its
batch b and its half of the query pixels (n), the full attention output
gamma * (V @ softmax(Q^T K)^T) + x over all m=4096 keys.

SPMD uniformity trick: the host permutes each core's pixel columns so that
the core's OWN query half always sits first. Attention is permutation-
invariant over keys, so every core runs the identical program.

Performance model notes (measured on this device/simulator):
  - PE matmul cost = output-free-size rows x clock; dtype/perf_mode do NOT
    change it. The PE clock is HAM-gated: it ramps 1.2 -> 2.4 GHz only under
    a sustained stream of matmuls whose CONTRACTION spans (nearly) all 128
    partitions; 8-row or 64-row streams never ramp.
  - Therefore every matmul here is padded to a 128-partition contraction:
    q/k are materialized as [128, n] tiles whose rows 8..127 are exact zeros
    (they fall out of zero weight columns, costing nothing), and the setup
    matmuls use zero-padded weight rows against "junk" x rows.
  - The scalar engine's exp stream (8.4M elements/core @ 1 elem/lane/cycle
    @1.2GHz) is the ~55us floor; per-instruction overhead ~253ns pushes it
    to ~67us with 3-PSUM-bank exp groups (GRP=3).

Per-core dataflow (biases folded into matmuls via a ones-row on the x
operand / a bias-row on the weight operand; x arrives in bf16 from host):
  1. k[128pad, m] / q[128pad, n] via [128, 128] zero-padded weight matmuls;
     psum -> bf16 SBUF copies (the pad rows copy over as exact zeros).
  2. vT[m, 65] = gamma*(x_blk.T @ Wv.T | bv) via 32 small matmuls, plus a
     memset ones column (softmax denominator).
  3. For each 512-wide query chunk: energy^T[m, n] = k^T q per 128-row
     m-block (PSUM, 128-deep padded contraction), exp on the scalar engine
     in GRP-bank groups (double buffered), then out_aug[65, n] += vT^T @
     expE accumulated over m-blocks. Row 64 of out_aug = softmax denom.
  4. Normalize: DVE reciprocal of the denominator row, gpsimd
     partition_broadcast, DVE multiply + residual add, DMA out.
"""

import os
import sys
import types
import numpy as np
import ml_dtypes


def _ensure_ntff_hook_importable():
    """bass_utils imports antenv.axon_hooks when tracing is requested via
    BASS_TRACE; some images lack that module. Provide it (backed by the
    ctypes hook from trn_boot when available, else a None hook, which
    bass_utils handles by skipping the trace)."""
    try:
        import antenv.axon_hooks  # noqa: F401
        return
    except ImportError:
        pass
    hook = None
    try:
        from trn_agent_boot.trn_boot import _ntff_profile_via_ctypes
        so = "/opt/axon/libaxon_pjrt.so"
        if os.path.exists(so):
            hook = _ntff_profile_via_ctypes(so)
    except Exception:
        hook = None
    mod = types.ModuleType("antenv.axon_hooks")
    mod.get_axon_ntff_profile_hook = lambda: hook
    sys.modules["antenv.axon_hooks"] = mod

B, C, H, W = 4, 64, 64, 64
N = H * W              # 4096 pixels
NHALF = N // 2         # 2048 query pixels per core
INTER = C // 8         # 8
NCORES = 8
MBLK = 128             # m-block (PSUM partition tile)
NCHUNK = 512           # query-chunk (PSUM bank free size)
NJ = N // MBLK         # 32 m-blocks
NT = NHALF // NCHUNK   # 4 query chunks

GRP = int(os.environ.get("KGRP", "3"))    # m-blocks (PSUM banks) per exp
EBUFS = int(os.environ.get("KEBUFS", "2"))  # energy tile double-buffering
NWARM = int(os.environ.get("KWARM", "8"))   # HAM warm-up matmuls during DMA
NWARM2 = int(os.environ.get("KWARM2", "10"))  # small tail warm-up matmuls

_compiled = {}
LAST_RESULT = None


def _group_sizes(j0, j1):
    # split [j0, j1) into groups of <= GRP, avoiding a trailing 1
    sizes = []
    left = j1 - j0
    while left > 0:
        g = min(GRP, left)
        if left - g == 1:
            g -= 1
        if g == 0:
            g = 1
        sizes.append(g)
        left -= g
    return sizes


def _build():
    import concourse.bacc as bacc
    import concourse.mybir as mybir
    from concourse.tile import TileContext

    dt = mybir.dt
    f32, bf16 = dt.float32, dt.bfloat16
    EXP = mybir.ActivationFunctionType.Exp

    nc = bacc.Bacc("TRN2", target_bir_lowering=False, debug=False,
                   num_devices=NCORES)

    # host-prepped inputs (see kernel() below)
    xa = nc.dram_tensor("xa", [128, NHALF], bf16, kind="ExternalInput").ap()
    xb2 = nc.dram_tensor("xb2", [128, NHALF], bf16, kind="ExternalInput").ap()
    xres = nc.dram_tensor("xres", [C, NHALF], f32, kind="ExternalInput").ap()
    wqk = nc.dram_tensor("wqk", [128, 256], bf16, kind="ExternalInput").ap()
    wv = nc.dram_tensor("wv_", [128, C], bf16, kind="ExternalInput").ap()
    out = nc.dram_tensor("out", [C, NHALF], f32, kind="ExternalOutput").ap()

    with TileContext(nc) as tc:
        with tc.tile_pool(name="const", bufs=1) as cp, \
             tc.tile_pool(name="eps", bufs=EBUFS, space="PSUM") as eps, \
             tc.tile_pool(name="ops", bufs=2, space="PSUM") as ops, \
             tc.tile_pool(name="work", bufs=int(os.environ.get("KWP", "3"))) as wp, \
             tc.tile_pool(name="fin", bufs=2) as fp:

            # ---- HAM warm-up: dense matmuls on a zeroed tile, issued with
            # no DMA dependency so they run during the initial DMA window.
            # By the time real work starts the PE clock is at 2.4 GHz. ----
            if NWARM > 0:
                wu = cp.tile([128, NCHUNK], bf16, tag="wu", name="wu")
                nc.vector.memset(wu[:, :], 0.0)
                wu_p = ops.tile([C, NCHUNK], f32, tag="o", name="wu_p")
                for _ in range(NWARM):
                    nc.tensor.matmul(wu_p[:, :], wu[:, 0:C], wu[:, :],
                                     start=True, stop=True)
                # fine-grained tail: keeps the PE busy across DMA-arrival
                # jitter without queue-delaying the first real matmul much
                for _ in range(NWARM2):
                    nc.tensor.matmul(wu_p[:, 0:128], wu[:, 0:C],
                                     wu[:, 0:128], start=True, stop=True)

            # DMA issue order matters: the first k/q matmul needs xa piece 1
            # + wqk, so those go first; wv (vT setup), xb2 (other half) and
            # xres (epilogue residual) are needed later.
            xa_t = cp.tile([128, NHALF], bf16, tag="xa", name="xa_t")
            nc.sync.dma_start(out=xa_t[:, 0:NCHUNK], in_=xa[:, 0:NCHUNK])
            wqk_t = cp.tile([128, 256], bf16, tag="wqk", name="wqk_t")
            nc.sync.dma_start(out=wqk_t[:, :], in_=wqk)
            nc.sync.dma_start(out=xa_t[:, NCHUNK:], in_=xa[:, NCHUNK:])
            wv_t = cp.tile([128, C], bf16, tag="wv", name="wv_t")
            nc.sync.dma_start(out=wv_t[:, :], in_=wv)
            xb_t = cp.tile([128, NHALF], bf16, tag="xb", name="xb_t")
            nc.sync.dma_start(out=xb_t[:, :], in_=xb2)
            xr_t = cp.tile([C, NHALF], f32, tag="xr", name="xr_t")
            nc.sync.dma_start(out=xr_t[:, :], in_=xres)

            q_t = cp.tile([128, NHALF], bf16, tag="q", name="q_t")
            k_t = cp.tile([128, N], bf16, tag="k", name="k_t")
            vt = cp.tile([128, NJ * (C + 1)], bf16, tag="vt", name="vt")
            vt3 = vt.rearrange("p (j c) -> p j c", c=C + 1)
            nc.vector.memset(vt3[:, :, C], 1.0)

            # ---- setup emitters (all 128-deep padded contractions) ----
            def emit_k(half, srct, t):
                # k rows land on psum partitions 0..7; 8..127 are exact
                # zeros from the zero weight columns.
                sl = slice(NCHUNK * (NT * half + t), NCHUNK * (NT * half + t + 1))
                kp = ops.tile([128, NCHUNK], f32, tag="o", name="kp")
                nc.tensor.matmul(kp[:, :], wqk_t[:, 0:128],
                                 srct[:, NCHUNK * t:NCHUNK * (t + 1)],
                                 start=True, stop=True)
                nc.vector.tensor_copy(k_t[:, sl], kp[:, :])

            def emit_q(t):
                sl = slice(NCHUNK * t, NCHUNK * (t + 1))
                qp = ops.tile([128, NCHUNK], f32, tag="o", name="qp")
                nc.tensor.matmul(qp[:, :], wqk_t[:, 128:256], xa_t[:, sl],
                                 start=True, stop=True)
                nc.vector.tensor_copy(q_t[:, sl], qp[:, :])

            def emit_vt(half, srct, j4):
                v_p = ops.tile([128, 4 * C], f32, tag="o", name="v_p")
                for jj in range(4):
                    jl = 4 * j4 + jj
                    nc.tensor.matmul(
                        v_p[:, C * jj:C * (jj + 1)],
                        srct[:, MBLK * jl:MBLK * (jl + 1)],
                        wv_t[:, :], start=True, stop=True)
                v_p4 = v_p.rearrange("p (j c) -> p j c", c=C)
                jg = 16 * half + 4 * j4
                nc.vector.tensor_copy(vt3[:, jg:jg + 4, 0:C], v_p4)

            # The AV batch for a group is deferred by one group in program
            # order: the PE queue then holds the NEXT group's energy matmuls
            # ahead of the previous group's AVs, so exp never waits on the
            # AV tail (important at chunk boundaries).
            pend_av = []

            def flush_av():
                while pend_av:
                    flush_one_av()

            # setup work not needed before chunk 0's first group is drip-fed
            # between groups (2 pieces per group) so the PE never inserts a
            # long setup batch between energy groups (which would starve exp)
            setup_thunks = []

            def flush_one_av():
                oa_p, ex_p, w_p, j_p, g_p = pend_av.pop(0)
                for jj in range(g_p):
                    nc.tensor.matmul(oa_p[:, :], vt3[:, j_p + jj, :],
                                     ex_p[:, w_p * jj:w_p * (jj + 1)],
                                     start=(j_p + jj == 0),
                                     stop=(j_p + jj == NJ - 1))

            def emit_groups(oa, j0, j1, col0, width, grp):
                # energy + exp + (lagged) AV for m-blocks [j0, j1) over query
                # columns [col0, col0+width); grp m-blocks per exp
                # instruction. The first group of a window is kept small so
                # exp restarts with minimal pipeline-fill latency.
                q_rhs = q_t[:, col0:col0 + width]
                j = j0
                left = j1 - j0
                it_idx = 0
                while left > 0:
                    g = 2 if (j0 == 0 and it_idx == 0 and left > grp) \
                        else min(grp, left)
                    if left - g == 1:
                        g -= 1
                    if g == 0:
                        g = 1
                    e = eps.tile([128, NCHUNK * GRP], f32, tag="e", name="e")
                    for jj in range(g):
                        k_lhs = k_t[:, MBLK * (j + jj):MBLK * (j + jj + 1)]
                        nc.tensor.matmul(
                            e[:, width * jj:width * (jj + 1)],
                            k_lhs, q_rhs, start=True, stop=True)
                    ex = wp.tile([128, NCHUNK * GRP], bf16, tag="ex", name="ex")
                    nc.scalar.activation(ex[:, 0:width * g],
                                         e[:, 0:width * g], EXP)
                    pend_av.append((oa, ex, width, j, g))
                    # lag-1 in steady state; at a window start, hold the
                    # previous window's last AV batch two groups longer so
                    # this window's first two energy groups reach the PE
                    # queue ahead of it (exp restarts without a bubble)
                    if j0 == 0 and it_idx < 2:
                        pass
                    else:
                        while len(pend_av) > 1:
                            flush_one_av()
                    for _ in range(2):
                        if setup_thunks:
                            setup_thunks.pop(0)()
                    j += g
                    left -= g
                    it_idx += 1

            def epilogue(oa, col0, width):
                # normalize + residual + store (PE-free), two pipelined
                # halves. Both reciprocals are emitted first so they run
                # back-to-back on the DVE instead of the second one queueing
                # behind half 0's multiply/add. (divide / custom-DVE fast
                # reciprocal are unsupported by this runtime's codegen.)
                nparts = 2
                HC = width // nparts
                recs = []
                for hh in range(nparts):
                    hs = slice(HC * hh, HC * (hh + 1))
                    rec = fp.tile([1, HC], f32, tag=f"rec{hh}", name="rec")
                    nc.vector.reciprocal(rec[:, :], oa[C:C + 1, hs])
                    recs.append(rec)
                for hh in range(nparts):
                    hs = slice(HC * hh, HC * (hh + 1))
                    gs = slice(col0 + HC * hh, col0 + HC * (hh + 1))
                    bcs = fp.tile([C, HC], f32, tag=f"bcs{hh}", name="bcs")
                    nc.gpsimd.partition_broadcast(bcs[:, :], recs[hh][:, :])
                    t1 = fp.tile([C, HC], f32, tag=f"t1{hh}", name="t1")
                    nc.vector.tensor_mul(t1[:, :], oa[0:C, hs], bcs[:, :])
                    fin = fp.tile([C, HC], f32, tag=f"fin{hh}", name="fin")
                    nc.vector.tensor_add(fin[:, :], t1[:, :], xr_t[:, gs])
                    nc.sync.dma_start(out=out[:, gs], in_=fin[:, :])

            # ---- setup: only what window 0's first group needs runs
            # up-front (q first: its copy is on the first-exp critical
            # path); the rest drips in between groups, 2 pieces per group.
            # q for the later windows is deferred into window 1's slack. ----
            emit_q(0)
            emit_k(0, xa_t, 0)
            emit_vt(0, xa_t, 0)
            setup_thunks.extend([
                lambda: emit_k(0, xa_t, 1), lambda: emit_vt(0, xa_t, 1),
                lambda: emit_k(0, xa_t, 2), lambda: emit_vt(0, xa_t, 2),
                lambda: emit_k(0, xa_t, 3), lambda: emit_vt(0, xa_t, 3),
                lambda: emit_k(1, xb_t, 0), lambda: emit_vt(1, xb_t, 0),
                lambda: emit_k(1, xb_t, 1), lambda: emit_vt(1, xb_t, 1),
                lambda: emit_k(1, xb_t, 2), lambda: emit_vt(1, xb_t, 2),
                lambda: emit_k(1, xb_t, 3), lambda: emit_vt(1, xb_t, 3),
                lambda: emit_q(1), lambda: emit_q(2), lambda: emit_q(3),
            ])
            # query-column windows: three 512-wide chunks, then two 256-wide
            # parts (with 2*GRP m-blocks per exp: same ACT efficiency) so the
            # final epilogue tail works on half-size pieces
            windows = [(NCHUNK * t, NCHUNK) for t in range(NT - 1)]
            windows += [(NCHUNK * (NT - 1), NCHUNK // 2),
                        (NCHUNK * (NT - 1) + NCHUNK // 2, NCHUNK // 2)]

            oa_list = []
            for wi, (col0, width) in enumerate(windows):
                grp = GRP * NCHUNK // width
                oa = ops.tile([C + 1, width], f32, tag="o",
                              name=f"oa{wi}")
                oa_list.append((oa, col0, width))
                if wi == 0:
                    emit_groups(oa, 0, 32, col0, width, grp)
                else:
                    emit_groups(oa, 0, 8, col0, width, grp)
                    # the previous window's accumulator must be fully
                    # written before its epilogue reads it
                    while pend_av and pend_av[0][0] is not oa:
                        flush_one_av()
                    epilogue(*oa_list[wi - 1])
                    emit_groups(oa, 8, 32, col0, width, grp)
            flush_av()
            epilogue(*oa_list[-1])

    nc.compile()
    return nc


def _get_compiled():
    if "nc" not in _compiled:
        _compiled["nc"] = _build()
    return _compiled["nc"]


def kernel(x, Wq, bq, Wk, bk, Wv, bv, gamma):
    global LAST_RESULT
    _ensure_ntff_hook_importable()
    from concourse.bass_utils import run_bass_kernel_spmd

    nc = _get_compiled()

    x = np.asarray(x, dtype=np.float32)
    xf = x.reshape(B, C, N)
    Wq, Wk, Wv = np.asarray(Wq), np.asarray(Wk), np.asarray(Wv)
    bq, bk, bv = np.asarray(bq), np.asarray(bk), np.asarray(bv)
    gval = float(np.asarray(gamma).reshape(-1)[0])

    # [128, 128] weight blocks: rows 0..63 = W.T, row 64 = bias, the rest
    # zeros. The zero columns 8..127 make the q/k psum pad rows exact zeros.
    def aug_qk(wT, bias):
        a = np.zeros((128, 128), np.float32)
        a[0:C, 0:INTER] = wT
        a[C, 0:INTER] = bias
        return a

    wqk_a = np.concatenate(
        [aug_qk(Wk.T, bk), aug_qk(Wq.T, bq)], axis=1
    ).astype(ml_dtypes.bfloat16)
    wv_a = np.zeros((128, C), np.float32)
    wv_a[0:C] = gval * Wv.T
    wv_a[C] = gval * bv
    wv_a = wv_a.astype(ml_dtypes.bfloat16)

    ones = np.ones((1, NHALF), dtype=np.float32)
    in_maps = []
    for core in range(NCORES):
        b, h = divmod(core, 2)
        own = xf[b][:, h * NHALF:(h + 1) * NHALF]
        oth = xf[b][:, (1 - h) * NHALF:(2 - h) * NHALF]
        # rows 65..127 are junk-fill (must be finite; they multiply zero
        # weight rows)
        xa_core = np.concatenate([own, ones, oth[0:63]],
                                 axis=0).astype(ml_dtypes.bfloat16)
        xb_core = np.concatenate([oth, ones, own[0:63]],
                                 axis=0).astype(ml_dtypes.bfloat16)
        in_maps.append({
            "xa": np.ascontiguousarray(xa_core),
            "xb2": np.ascontiguousarray(xb_core),
            "xres": np.ascontiguousarray(own, dtype=np.float32),
            "wqk": np.ascontiguousarray(wqk_a),
            "wv_": np.ascontiguousarray(wv_a),
        })

    trace = bool(os.environ.get("KTRACE"))
    res = run_bass_kernel_spmd(nc, in_maps, list(range(NCORES)), trace=trace)
    LAST_RESULT = res

    outf = np.empty((B, C, N), dtype=np.float32)
    for core in range(NCORES):
        b, h = divmod(core, 2)
        outf[b][:, h * NHALF:(h + 1) * NHALF] = res.results[core]["out"]
    return outf.reshape(B, C, H, W)


# revision 32
# speedup vs baseline: 1.2316x; 1.2316x over previous
"""Trainium2 Bass kernel for nn_AttentionBlock (B=4, C=64, H=W=64, INTER=8).

Sharding: 8 cores = 4 batches x 2 query-halves. Each core computes, for its
batch b and its half of the query pixels (n), the full attention output
gamma * (V @ softmax(Q^T K)^T) + x over all m=4096 keys.

SPMD uniformity trick: the host permutes each core's pixel columns so that
the core's OWN query half always sits first. Attention is permutation-
invariant over keys, so every core runs the identical program.

Performance model notes (measured on this device/simulator):
  - PE matmul cost = output-free-size rows x clock; dtype/perf_mode do NOT
    change it. The PE clock is HAM-gated: it ramps 1.2 -> 2.4 GHz only under
    a sustained stream of matmuls whose CONTRACTION spans (nearly) all 128
    partitions; 8-row or 64-row streams never ramp.
  - Therefore every matmul here is padded to a 128-partition contraction:
    q/k are materialized as [128, n] tiles whose rows 8..127 are exact zeros
    (they fall out of zero weight columns, costing nothing), and the setup
    matmuls use zero-padded weight rows against "junk" x rows.
  - The scalar engine's exp stream (8.4M elements/core @ 1 elem/lane/cycle
    @1.2GHz) is the ~55us floor; per-instruction overhead ~253ns pushes it
    to ~67us with 3-PSUM-bank exp groups (GRP=3).

Per-core dataflow (biases folded into matmuls via a ones-row on the x
operand / a bias-row on the weight operand; x arrives in bf16 from host):
  1. k[128pad, m] / q[128pad, n] via [128, 128] zero-padded weight matmuls;
     psum -> bf16 SBUF copies (the pad rows copy over as exact zeros).
  2. vT[m, 65] = gamma*(x_blk.T @ Wv.T | bv) via 32 small matmuls, plus a
     memset ones column (softmax denominator).
  3. For each 512-wide query chunk: energy^T[m, n] = k^T q per 128-row
     m-block (PSUM, 128-deep padded contraction), exp on the scalar engine
     in GRP-bank groups (double buffered), then out_aug[65, n] += vT^T @
     expE accumulated over m-blocks. Row 64 of out_aug = softmax denom.
  4. Normalize: DVE reciprocal of the denominator row, gpsimd
     partition_broadcast, DVE multiply + residual add, DMA out.
"""

import os
import sys
import types
import numpy as np
import ml_dtypes


def _ensure_ntff_hook_importable():
    """bass_utils imports antenv.axon_hooks when tracing is requested via
    BASS_TRACE; some images lack that module. Provide it (backed by the
    ctypes hook from trn_boot when available, else a None hook, which
    bass_utils handles by skipping the trace)."""
    try:
        import antenv.axon_hooks  # noqa: F401
        return
    except ImportError:
        pass
    hook = None
    try:
        from trn_agent_boot.trn_boot import _ntff_profile_via_ctypes
        so = "/opt/axon/libaxon_pjrt.so"
        if os.path.exists(so):
            hook = _ntff_profile_via_ctypes(so)
    except Exception:
        hook = None
    mod = types.ModuleType("antenv.axon_hooks")
    mod.get_axon_ntff_profile_hook = lambda: hook
    sys.modules["antenv.axon_hooks"] = mod

B, C, H, W = 4, 64, 64, 64
N = H * W              # 4096 pixels
NHALF = N // 2         # 2048 query pixels per core
INTER = C // 8         # 8
NCORES = 8
MBLK = 128             # m-block (PSUM partition tile)
NCHUNK = 512           # query-chunk (PSUM bank free size)
NJ = N // MBLK         # 32 m-blocks
NT = NHALF // NCHUNK   # 4 query chunks

GRP = int(os.environ.get("KGRP", "3"))    # m-blocks (PSUM banks) per exp
EBUFS = int(os.environ.get("KEBUFS", "2"))  # energy tile double-buffering
NWARM = int(os.environ.get("KWARM", "8"))   # HAM warm-up matmuls during DMA
NWARM2 = int(os.environ.get("KWARM2", "10"))  # small tail warm-up matmuls

_compiled = {}
LAST_RESULT = None


def _group_sizes(j0, j1):
    # split [j0, j1) into groups of <= GRP, avoiding a trailing 1
    sizes = []
    left = j1 - j0
    while left > 0:
        g = min(GRP, left)
        if left - g == 1:
            g -= 1
        if g == 0:
            g = 1
        sizes.append(g)
        left -= g
    return sizes


def _build():
    import concourse.bacc as bacc
    import concourse.mybir as mybir
    from concourse.tile import TileContext

    dt = mybir.dt
    f32, bf16 = dt.float32, dt.bfloat16
    EXP = mybir.ActivationFunctionType.Exp

    nc = bacc.Bacc("TRN2", target_bir_lowering=False, debug=False,
                   num_devices=NCORES)

    # host-prepped inputs (see kernel() below)
    xa = nc.dram_tensor("xa", [128, NHALF], bf16, kind="ExternalInput").ap()
    xb2 = nc.dram_tensor("xb2", [128, NHALF], bf16, kind="ExternalInput").ap()
    xres = nc.dram_tensor("xres", [C, NHALF], f32, kind="ExternalInput").ap()
    wqk = nc.dram_tensor("wqk", [128, 256], bf16, kind="ExternalInput").ap()
    wv = nc.dram_tensor("wv_", [128, C], bf16, kind="ExternalInput").ap()
    out = nc.dram_tensor("out", [C, NHALF], f32, kind="ExternalOutput").ap()

    with TileContext(nc) as tc:
        with tc.tile_pool(name="const", bufs=1) as cp, \
             tc.tile_pool(name="eps", bufs=EBUFS, space="PSUM") as eps, \
             tc.tile_pool(name="ops", bufs=2, space="PSUM") as ops, \
             tc.tile_pool(name="work", bufs=int(os.environ.get("KWP", "3"))) as wp, \
             tc.tile_pool(name="fin", bufs=2) as fp:

            # ---- HAM warm-up: dense matmuls on a zeroed tile, issued with
            # no DMA dependency so they run during the initial DMA window.
            # By the time real work starts the PE clock is at 2.4 GHz. ----
            if NWARM > 0:
                wu = cp.tile([128, NCHUNK], bf16, tag="wu", name="wu")
                nc.vector.memset(wu[:, :], 0.0)
                wu_p = ops.tile([C, NCHUNK], f32, tag="o", name="wu_p")
                for _ in range(NWARM):
                    nc.tensor.matmul(wu_p[:, :], wu[:, 0:C], wu[:, :],
                                     start=True, stop=True)
                # fine-grained tail: keeps the PE busy across DMA-arrival
                # jitter without queue-delaying the first real matmul much
                for _ in range(NWARM2):
                    nc.tensor.matmul(wu_p[:, 0:128], wu[:, 0:C],
                                     wu[:, 0:128], start=True, stop=True)

            # DMA issue order matters: the first k/q matmul needs xa piece 1
            # + wqk, so those go first; wv (vT setup), xb2 (other half) and
            # xres (epilogue residual) are needed later.
            xa_t = cp.tile([128, NHALF], bf16, tag="xa", name="xa_t")
            nc.sync.dma_start(out=xa_t[:, 0:NCHUNK], in_=xa[:, 0:NCHUNK])
            wqk_t = cp.tile([128, 256], bf16, tag="wqk", name="wqk_t")
            nc.sync.dma_start(out=wqk_t[:, :], in_=wqk)
            nc.sync.dma_start(out=xa_t[:, NCHUNK:], in_=xa[:, NCHUNK:])
            wv_t = cp.tile([128, C], bf16, tag="wv", name="wv_t")
            nc.sync.dma_start(out=wv_t[:, :], in_=wv)
            xb_t = cp.tile([128, NHALF], bf16, tag="xb", name="xb_t")
            nc.sync.dma_start(out=xb_t[:, :], in_=xb2)
            xr_t = cp.tile([C, NHALF], f32, tag="xr", name="xr_t")
            nc.sync.dma_start(out=xr_t[:, :], in_=xres)

            q_t = cp.tile([128, NHALF], bf16, tag="q", name="q_t")
            k_t = cp.tile([128, N], bf16, tag="k", name="k_t")
            vt = cp.tile([128, NJ * (C + 1)], bf16, tag="vt", name="vt")
            vt3 = vt.rearrange("p (j c) -> p j c", c=C + 1)
            nc.vector.memset(vt3[:, :, C], 1.0)

            # ---- setup emitters (all 128-deep padded contractions) ----
            def emit_k(half, srct, t):
                # k rows land on psum partitions 0..7; 8..127 are exact
                # zeros from the zero weight columns.
                sl = slice(NCHUNK * (NT * half + t), NCHUNK * (NT * half + t + 1))
                kp = ops.tile([128, NCHUNK], f32, tag="o", name="kp")
                nc.tensor.matmul(kp[:, :], wqk_t[:, 0:128],
                                 srct[:, NCHUNK * t:NCHUNK * (t + 1)],
                                 start=True, stop=True)
                nc.vector.tensor_copy(k_t[:, sl], kp[:, :])

            def emit_q(t):
                sl = slice(NCHUNK * t, NCHUNK * (t + 1))
                qp = ops.tile([128, NCHUNK], f32, tag="o", name="qp")
                nc.tensor.matmul(qp[:, :], wqk_t[:, 128:256], xa_t[:, sl],
                                 start=True, stop=True)
                nc.vector.tensor_copy(q_t[:, sl], qp[:, :])

            def emit_vt(half, srct, j4):
                v_p = ops.tile([128, 4 * C], f32, tag="o", name="v_p")
                for jj in range(4):
                    jl = 4 * j4 + jj
                    nc.tensor.matmul(
                        v_p[:, C * jj:C * (jj + 1)],
                        srct[:, MBLK * jl:MBLK * (jl + 1)],
                        wv_t[:, :], start=True, stop=True)
                v_p4 = v_p.rearrange("p (j c) -> p j c", c=C)
                jg = 16 * half + 4 * j4
                nc.vector.tensor_copy(vt3[:, jg:jg + 4, 0:C], v_p4)

            # The AV batch for a group is deferred by one group in program
            # order: the PE queue then holds the NEXT group's energy matmuls
            # ahead of the previous group's AVs, so exp never waits on the
            # AV tail (important at chunk boundaries).
            pend_av = []

            def flush_av():
                while pend_av:
                    flush_one_av()

            # setup work not needed before chunk 0's first group is drip-fed
            # between groups (2 pieces per group) so the PE never inserts a
            # long setup batch between energy groups (which would starve exp)
            setup_thunks = []

            def flush_one_av():
                oa_p, ex_p, w_p, j_p, g_p = pend_av.pop(0)
                for jj in range(g_p):
                    nc.tensor.matmul(oa_p[:, :], vt3[:, j_p + jj, :],
                                     ex_p[:, w_p * jj:w_p * (jj + 1)],
                                     start=(j_p + jj == 0),
                                     stop=(j_p + jj == NJ - 1))

            def emit_groups(oa, j0, j1, col0, width, grp):
                # energy + exp + (lagged) AV for m-blocks [j0, j1) over query
                # columns [col0, col0+width); grp m-blocks per exp
                # instruction. The first group of a window is kept small so
                # exp restarts with minimal pipeline-fill latency.
                q_rhs = q_t[:, col0:col0 + width]
                j = j0
                left = j1 - j0
                it_idx = 0
                while left > 0:
                    g = 2 if (j0 == 0 and it_idx == 0 and left > grp) \
                        else min(grp, left)
                    if left - g == 1:
                        g -= 1
                    if g == 0:
                        g = 1
                    e = eps.tile([128, NCHUNK * GRP], f32, tag="e", name="e")
                    for jj in range(g):
                        k_lhs = k_t[:, MBLK * (j + jj):MBLK * (j + jj + 1)]
                        nc.tensor.matmul(
                            e[:, width * jj:width * (jj + 1)],
                            k_lhs, q_rhs, start=True, stop=True)
                    ex = wp.tile([128, NCHUNK * GRP], bf16, tag="ex", name="ex")
                    nc.scalar.activation(ex[:, 0:width * g],
                                         e[:, 0:width * g], EXP)
                    pend_av.append((oa, ex, width, j, g))
                    # lag-1 in steady state; at a window start, hold the
                    # previous window's last AV batch two groups longer so
                    # this window's first two energy groups reach the PE
                    # queue ahead of it (exp restarts without a bubble)
                    if j0 == 0 and it_idx < 2:
                        pass
                    else:
                        while len(pend_av) > 1:
                            flush_one_av()
                    for _ in range(2):
                        if setup_thunks:
                            setup_thunks.pop(0)()
                    j += g
                    left -= g
                    it_idx += 1

            def epilogue(oa, col0, width):
                # normalize + residual + store (PE-free), two pipelined
                # halves. Both reciprocals are emitted first so they run
                # back-to-back on the DVE instead of the second one queueing
                # behind half 0's multiply/add. (divide / custom-DVE fast
                # reciprocal are unsupported by this runtime's codegen.)
                nparts = 2
                HC = width // nparts
                recs = []
                for hh in range(nparts):
                    hs = slice(HC * hh, HC * (hh + 1))
                    rec = fp.tile([1, HC], f32, tag=f"rec{hh}", name="rec")
                    nc.vector.reciprocal(rec[:, :], oa[C:C + 1, hs])
                    recs.append(rec)
                for hh in range(nparts):
                    hs = slice(HC * hh, HC * (hh + 1))
                    gs = slice(col0 + HC * hh, col0 + HC * (hh + 1))
                    bcs = fp.tile([C, HC], f32, tag=f"bcs{hh}", name="bcs")
                    nc.gpsimd.partition_broadcast(bcs[:, :], recs[hh][:, :])
                    t1 = fp.tile([C, HC], f32, tag=f"t1{hh}", name="t1")
                    nc.vector.tensor_mul(t1[:, :], oa[0:C, hs], bcs[:, :])
                    fin = fp.tile([C, HC], f32, tag=f"fin{hh}", name="fin")
                    nc.vector.tensor_add(fin[:, :], t1[:, :], xr_t[:, gs])
                    nc.sync.dma_start(out=out[:, gs], in_=fin[:, :])

            # ---- setup: only what window 0's first group needs runs
            # up-front (q first: its copy is on the first-exp critical
            # path); the rest drips in between groups, 2 pieces per group.
            # q for the later windows is deferred into window 1's slack. ----
            emit_q(0)
            emit_k(0, xa_t, 0)
            emit_vt(0, xa_t, 0)
            setup_thunks.extend([
                lambda: emit_k(0, xa_t, 1), lambda: emit_vt(0, xa_t, 1),
                lambda: emit_k(0, xa_t, 2), lambda: emit_vt(0, xa_t, 2),
                lambda: emit_k(0, xa_t, 3), lambda: emit_vt(0, xa_t, 3),
                lambda: emit_k(1, xb_t, 0), lambda: emit_vt(1, xb_t, 0),
                lambda: emit_k(1, xb_t, 1), lambda: emit_vt(1, xb_t, 1),
                lambda: emit_k(1, xb_t, 2), lambda: emit_vt(1, xb_t, 2),
                lambda: emit_k(1, xb_t, 3), lambda: emit_vt(1, xb_t, 3),
                lambda: emit_q(1), lambda: emit_q(2), lambda: emit_q(3),
            ])
            # query-column windows: three 512-wide chunks, then two 256-wide
            # parts (with 2*GRP m-blocks per exp: same ACT efficiency) so the
            # final epilogue tail works on half-size pieces
            windows = [(NCHUNK * t, NCHUNK) for t in range(NT - 1)]
            windows += [(NCHUNK * (NT - 1), NCHUNK // 2),
                        (NCHUNK * (NT - 1) + NCHUNK // 2, NCHUNK // 2)]

            oa_list = []
            for wi, (col0, width) in enumerate(windows):
                grp = GRP * NCHUNK // width
                oa = ops.tile([C + 1, width], f32, tag="o",
                              name=f"oa{wi}")
                oa_list.append((oa, col0, width))
                if wi == 0:
                    emit_groups(oa, 0, 32, col0, width, grp)
                else:
                    emit_groups(oa, 0, 8, col0, width, grp)
                    # the previous window's accumulator must be fully
                    # written before its epilogue reads it
                    while pend_av and pend_av[0][0] is not oa:
                        flush_one_av()
                    epilogue(*oa_list[wi - 1])
                    emit_groups(oa, 8, 32, col0, width, grp)
            flush_av()
            epilogue(*oa_list[-1])

    nc.compile()
    return nc


def _get_compiled():
    if "nc" not in _compiled:
        _compiled["nc"] = _build()
    return _compiled["nc"]


def kernel(x, Wq, bq, Wk, bk, Wv, bv, gamma):
    global LAST_RESULT
    _ensure_ntff_hook_importable()
    from concourse.bass_utils import run_bass_kernel_spmd

    nc = _get_compiled()

    x = np.asarray(x, dtype=np.float32)
    xf = x.reshape(B, C, N)
    Wq, Wk, Wv = np.asarray(Wq), np.asarray(Wk), np.asarray(Wv)
    bq, bk, bv = np.asarray(bq), np.asarray(bk), np.asarray(bv)
    gval = float(np.asarray(gamma).reshape(-1)[0])

    # [128, 128] weight blocks: rows 0..63 = W.T, row 64 = bias, the rest
    # zeros. The zero columns 8..127 make the q/k psum pad rows exact zeros.
    def aug_qk(wT, bias):
        a = np.zeros((128, 128), np.float32)
        a[0:C, 0:INTER] = wT
        a[C, 0:INTER] = bias
        return a

    wqk_a = np.concatenate(
        [aug_qk(Wk.T, bk), aug_qk(Wq.T, bq)], axis=1
    ).astype(ml_dtypes.bfloat16)
    wv_a = np.zeros((128, C), np.float32)
    wv_a[0:C] = gval * Wv.T
    wv_a[C] = gval * bv
    wv_a = wv_a.astype(ml_dtypes.bfloat16)

    ones = np.ones((1, NHALF), dtype=np.float32)
    in_maps = []
    for core in range(NCORES):
        b, h = divmod(core, 2)
        own = xf[b][:, h * NHALF:(h + 1) * NHALF]
        oth = xf[b][:, (1 - h) * NHALF:(2 - h) * NHALF]
        # rows 65..127 are junk-fill (must be finite; they multiply zero
        # weight rows)
        xa_core = np.concatenate([own, ones, oth[0:63]],
                                 axis=0).astype(ml_dtypes.bfloat16)
        xb_core = np.concatenate([oth, ones, own[0:63]],
                                 axis=0).astype(ml_dtypes.bfloat16)
        in_maps.append({
            "xa": np.ascontiguousarray(xa_core),
            "xb2": np.ascontiguousarray(xb_core),
            "xres": np.ascontiguousarray(own, dtype=np.float32),
            "wqk": np.ascontiguousarray(wqk_a),
            "wv_": np.ascontiguousarray(wv_a),
        })

    trace = bool(os.environ.get("KTRACE"))
    res = run_bass_kernel_spmd(nc, in_maps, list(range(NCORES)), trace=trace)
    LAST_RESULT = res

    outf = np.empty((B, C, N), dtype=np.float32)
    for core in range(NCORES):
        b, h = divmod(core, 2)
        outf[b][:, h * NHALF:(h + 1) * NHALF] = res.results[core]["out"]
    return outf.reshape(B, C, H, W)
